# revision 21
# baseline (speedup 1.0000x reference)
"""Trainium2 Bass kernel for multi-head self-attention (dense transformer block).

Problem: x[4, 2048, 1024], w_qkv[3072, 1024], w_out[1024, 1024], b_out[1024]
  qkv = x @ w_qkv.T, rearranged 'b t (d k h) -> k b h t d' (k=3, h=16)
  attn = softmax(q @ k.T * DIM**-0.5); out = (attn @ v) concat heads @ w_out.T + b_out

Sharding (8 cores): data-parallel over batch b (4) x tensor-parallel over
head-groups (2 groups of 8 heads).  Each core gets x[b] (pre-transposed on
host), the w_qkv rows for its 8 heads (pre-gathered/transposed on host), and
the matching w_out columns; it produces a partial [T, DIM] output which the
host sums per batch pair (the "all-reduce" of the row-sharded w_out matmul)
and adds b_out.

Device-side dataflow per core (all matmuls bf16; inputs cast to bf16 on the
host, halving input DMA):
  - Fused schedule: only kT[ob=0] and qT[ic=0] precede the first scores
    unit, so ScalarE's exp stream (the ~255us critical resource) starts
    ~15us in; the rest of kT, all of V, and later qT chunks drain between
    the early units while ScalarE still has exp backlog.
  - Per (i-chunk of 256 queries, head pair): S^T[j, i] = kT.T @ qT (two
    K=64 heads on PE row groups 0-63/64-127), exp on ScalarE with the
    softmax scale folded in, then out[i, 65] = expST.T @ [v | 1]
    accumulating over j -- the unnormalized attention output and its
    softmax denominator in one matmul; normalize with a per-partition
    reciprocal multiply.  The N=65 AV matmuls are LDWEIGHTS-bound
    (~53ns weight load vs ~27ns stream), so they run AV_DELAY units
    behind scores and are interleaved 2-per-scores-matmul, hiding their
    weight loads under the scores streams' weight-port slack.
  - attn [t, o] -> [o, t] via DMA-xbar transposes (no PE time), project
    with w_out.T (bf16), DMA partial fp32 outputs from SBUF.
"""

import math
from contextlib import ExitStack
from dataclasses import dataclass

import numpy as np

import concourse.bass as bass
import concourse.mybir as mybir
import concourse.tile as tile
from concourse import bacc
from concourse.bass_utils import run_bass_kernel_spmd

F32 = mybir.dt.float32
F32R = mybir.dt.float32r
BF16 = mybir.dt.bfloat16
NP_BF16 = mybir.dt.np(BF16)
P = 128


@dataclass(frozen=True)
class Cfg:
    T: int = 2048      # sequence length
    DIM: int = 1024    # model dim (= qkv contraction dim)
    NH: int = 8        # heads per core
    DH: int = 64       # head dim
    SCALE: float = 1024.0 ** -0.5
    AV_DELAY: int = 4  # units the AV matmul trails its scores unit by

    @property
    def CB(self):      # contraction blocks of 128 over DIM
        return self.DIM // P

    @property
    def OD(self):      # per-core attention width = NH*DH
        return self.NH * self.DH

    @property
    def OB(self):      # o-blocks of 128 (= head pairs, 2 x 64)
        return self.OD // P

    @property
    def JB(self):      # key blocks of 128
        return self.T // P

    @property
    def ICSZ(self):    # query chunk size
        return min(256, self.T)

    @property
    def NIC(self):     # number of query chunks
        return self.T // self.ICSZ

    @property
    def IB(self):      # query blocks of 128 per chunk
        return self.ICSZ // P

    @property
    def TCH(self):     # t-chunk for phase-1 rhs streaming
        return min(256, self.T)

    @property
    def NTCH(self):
        return self.T // self.TCH

    @property
    def OCC(self):     # output-column chunk for the final projection
        return min(512, self.DIM)

    @property
    def NOCC(self):
        return self.DIM // self.OCC


def _emit_kernel(tc, cfg, xT, wq, wk, wv, woT, out):
    """Emit the per-core attention kernel under an open TileContext."""
    nc = tc.nc
    c = cfg
    VW = c.DH + 1  # per-head V width incl. ones column
    AVD = c.AV_DELAY

    ctx = ExitStack()
    with ctx:
        persist = ctx.enter_context(tc.tile_pool(name="persist", bufs=1))
        mmp = ctx.enter_context(tc.tile_pool(name="mmp", bufs=2, space="PSUM"))
        smp = ctx.enter_context(tc.tile_pool(name="smp", bufs=4, space="PSUM"))
        xp = ctx.enter_context(tc.tile_pool(name="xp", bufs=c.NTCH))
        wp = ctx.enter_context(tc.tile_pool(name="wp", bufs=1))
        ep = ctx.enter_context(tc.tile_pool(name="ep", bufs=2 * (AVD + 1)))
        ap = ctx.enter_context(tc.tile_pool(name="ap", bufs=2))
        atp = ctx.enter_context(tc.tile_pool(name="atp", bufs=2))
        op = ctx.enter_context(tc.tile_pool(name="op", bufs=2))
        rp = ctx.enter_context(tc.tile_pool(name="rp", bufs=2))

        qT_sb = persist.tile([P, c.OB, c.T], BF16, name="qT_sb", tag="qT")
        kT_sb = persist.tile([P, c.OB, c.T], BF16, name="kT_sb", tag="kT")
        v_sb = persist.tile([P, c.JB, c.NH, VW], BF16, name="v_sb", tag="v")
        woT_sb = persist.tile([P, c.OB, c.DIM], BF16, name="woT_sb", tag="woT")

        nc.gpsimd.memset(v_sb[:, :, :, c.DH : c.DH + 1], 1.0)

        xT_r = xT.rearrange("(cb p) t -> p cb t", p=P)
        wq_r = wq.rearrange("(cb p) o -> p cb o", p=P)
        wk_r = wk.rearrange("(cb p) o -> p cb o", p=P)
        wv_r = wv.rearrange("(cb p) o -> p cb o", p=P)

        # ---- input DMAs, all issued up front (x stays resident: 4MB bf16,
        # reused by the kT/V drain pieces AND the per-chunk qT projections) ----
        wk_cb = []
        for cb in range(c.CB):
            wk_t = wp.tile([P, c.OD], BF16, name=f"wk_{cb}", tag=f"wk{cb}")
            nc.sync.dma_start(out=wk_t, in_=wk_r[:, cb, :])
            wk_cb.append(wk_t)
        # x lands chunk-granular (one descriptor per [P, CB, TCH] chunk):
        # fewer SP-queue issue slots than per-cb tiles.  Order: wk, x0, wq,
        # x1.., so the first scores group (kT[ob0, jb0-3] + qT[ob0]) clears
        # after ~2 x-chunks instead of the whole 4MB of x.
        x_tiles = {}

        def x_dma(tch):
            tsl = bass.ts(tch, c.TCH)
            x_t = xp.tile([P, c.CB, c.TCH], BF16, name=f"x_{tch}", tag="x")
            nc.sync.dma_start(out=x_t, in_=xT_r[:, :, tsl])
            x_tiles[tch] = x_t

        x_dma(0)
        wq_sb = wp.tile([P, c.CB, c.OD], BF16, name="wq_sb", tag="wq")
        nc.sync.dma_start(out=wq_sb, in_=wq_r)
        for tch in range(1, c.NTCH):
            x_dma(tch)
        wv_sb = wp.tile([P, c.CB, c.OD], BF16, name="wv_sb", tag="wv")
        nc.sync.dma_start(out=wv_sb, in_=wv_r)
        nc.sync.dma_start(
            out=woT_sb, in_=woT.rearrange("(ob p) n -> p ob n", p=P)
        )

        def emit_kt(ob, tch):
            """kT[o-block ob, t-chunk tch] = wk[:, ob].T @ x[:, tch]."""
            ps = mmp.tile([P, c.TCH], F32, name="ps_qk", tag="mm")
            for cb in range(c.CB):
                nc.tensor.matmul(
                    ps,
                    wk_cb[cb][:, bass.ts(ob, P)],
                    x_tiles[tch][:, cb, :],
                    start=(cb == 0),
                    stop=(cb == c.CB - 1),
                )
            nc.vector.tensor_copy(out=kT_sb[:, ob, bass.ts(tch, c.TCH)], in_=ps)

        def emit_v(tch, tbl):
            """V[t-block] = x[:, tb].T @ wv (one 128-row block of V)."""
            ps_v = smp.tile([P, c.OD], F32, name="ps_v", tag="sm")
            for cb in range(c.CB):
                nc.tensor.matmul(
                    ps_v,
                    x_tiles[tch][:, cb, bass.ts(tbl, P)],
                    wv_sb[:, cb, :],
                    start=(cb == 0),
                    stop=(cb == c.CB - 1),
                )
            tb = tch * (c.TCH // P) + tbl
            nc.vector.tensor_copy(
                out=v_sb[:, tb, :, 0 : c.DH],
                in_=ps_v.rearrange("p (h d) -> p h d", h=c.NH),
            )

        def emit_q_ob(ic, ob):
            """One o-block piece of the qT projection for query chunk ic."""
            ps = smp.tile([P, c.TCH], F32, name="ps_q2", tag="sm")
            for cb in range(c.CB):
                nc.tensor.matmul(
                    ps,
                    wq_sb[:, cb, bass.ts(ob, P)],
                    x_tiles[ic][:, cb, :],
                    start=(cb == 0),
                    stop=(cb == c.CB - 1),
                )
            nc.vector.tensor_copy(out=qT_sb[:, ob, bass.ts(ic, c.TCH)], in_=ps)

        attn_tiles = {}
        attnT_tiles = {}

        def make_scores(ic, hp):
            """S^T then exp for head pair hp at query chunk ic.

            Returns (e_pair, generator).  The generator yields after each
            jb's pair of K=64 matmuls so the scheduler can interleave the
            LDWEIGHTS-heavy AV matmuls under the scores streams.
            """
            isl = bass.ts(ic, c.ICSZ)
            e_pair = []
            for half in range(2):
                e_pair.append(
                    ep.tile([P, c.JB, c.ICSZ], BF16,
                            name=f"e_{ic}_{hp}_{half}", tag="e")
                )
            # group JJ key-blocks per PSUM tile so each exp activation
            # covers FD = JJ*ICSZ = 1024 elements (amortizes ACT overhead)
            JJ = min(4096 // (4 * c.ICSZ), c.JB)

            def gen():
                for g in range(c.JB // JJ):
                    ps = []
                    for half in range(2):
                        ps.append(
                            mmp.tile([P, JJ, c.ICSZ], F32,
                                     name=f"ps_s{half}", tag="mm")
                        )
                    for jj in range(JJ):
                        jb = g * JJ + jj
                        for half in range(2):
                            rows = slice(64 * half, 64 * half + 64)
                            nc.tensor.matmul(
                                ps[half][:, jj, :],
                                kT_sb[rows, hp, bass.ts(jb, P)],
                                qT_sb[rows, hp, isl],
                                start=True,
                                stop=True,
                                tile_position=(64 * half, 0),
                            )
                        yield
                    for half in range(2):
                        nc.scalar.activation(
                            out=e_pair[half][:, g * JJ : (g + 1) * JJ, :],
                            in_=ps[half],
                            func=mybir.ActivationFunctionType.Exp,
                            scale=c.SCALE,
                        )

            return e_pair, gen()

        def make_av(ic, hp, e_pair):
            """attn[i, dh] = norm(expST.T @ [v|1]) for both heads of hp,
            then DMA-xbar-transpose this head pair's 128 columns.

            Returns a generator yielding every 2 matmuls: the N=65 AV
            matmuls are LDWEIGHTS-bound (~53ns vs 27ns stream), so the
            scheduler interleaves them under scores streams, whose weight
            port has ~54ns of slack per matmul.
            """
            if ic not in attn_tiles:
                attn_tiles[ic] = ap.tile(
                    [P, c.IB, c.OD], BF16, name=f"attn_{ic}", tag="attn"
                )
            attn_sb = attn_tiles[ic]

            def gen():
                for half in range(2):
                    h = 2 * hp + half
                    e = e_pair[half]
                    for ib in range(c.IB):
                        ps_av = smp.tile([P, c.OD], F32, name="ps_av", tag="sm")
                        for jb in range(c.JB):
                            nc.tensor.matmul(
                                ps_av[:, 0:VW],
                                e[:, jb, bass.ts(ib, P)],
                                v_sb[:, jb, h, :],
                                start=(jb == 0),
                                stop=(jb == c.JB - 1),
                            )
                            if jb % 2 == 1:
                                yield
                        rec = rp.tile([P, 1], F32, name="rec", tag="rec")
                        nc.vector.reciprocal(rec, ps_av[:, c.DH : c.DH + 1])
                        nc.vector.tensor_scalar_mul(
                            out=attn_sb[:, ib, bass.ts(h, c.DH)],
                            in0=ps_av[:, 0 : c.DH],
                            scalar1=rec,
                        )
                if ic not in attnT_tiles:
                    attnT_tiles[ic] = atp.tile(
                        [P, c.OB, c.ICSZ], BF16, name=f"attnT_{ic}", tag="attnT"
                    )
                attnT_sb = attnT_tiles[ic]
                for ib in range(c.IB):
                    nc.sync.dma_start_transpose(
                        out=attnT_sb[:, hp, bass.ts(ib, P)],
                        in_=attn_sb[:, ib, bass.ts(hp, P)],
                    )
                if hp == c.OB - 1:
                    attn_tiles.pop(ic)

            return gen()

        def emit_finalize(ic, half):
            """One half of the output projection for a finished chunk,
            split so the PE burst stays smaller than ScalarE's slack."""
            attnT_sb = attnT_tiles[ic]
            h_tb = c.IB // 2
            for tb in range(half * h_tb, (half + 1) * h_tb):
                for occ in range(c.NOCC):
                    ps_o = smp.tile([P, c.OCC], F32, name="ps_o", tag="sm")
                    for ob in range(c.OB):
                        nc.tensor.matmul(
                            ps_o,
                            attnT_sb[:, ob, bass.ts(tb, P)],
                            woT_sb[:, ob, bass.ts(occ, c.OCC)],
                            start=(ob == 0),
                            stop=(ob == c.OB - 1),
                        )
                    o_sb = op.tile([P, c.OCC], F32, name="o_sb", tag="ost")
                    nc.vector.tensor_copy(out=o_sb, in_=ps_o)
                    t0 = ic * c.ICSZ + tb * P
                    nc.sync.dma_start(
                        out=out[t0 : t0 + P, bass.ts(occ, c.OCC)],
                        in_=o_sb,
                    )
            if half == 1:
                attnT_tiles.pop(ic)

        # ---- schedule ----
        # Fused phases: only kT[ob=0] + qT[ic=0] are computed before the
        # first scores unit, so ScalarE's exp stream starts ~40us earlier.
        # The rest of kT, and all of V, drain between the early units while
        # ScalarE still has exp backlog; AV is software-pipelined AVD units
        # behind scores so its V dependency is ready when it fires.
        assert c.ICSZ == c.TCH and c.NIC == c.NTCH
        # phase A: just enough for the first scores GROUP -- kT[ob0] for
        # jb 0-3 (x chunks 0-1) and qT[ob0].  The rest of kT[ob0] is
        # emitted between unit 0's score groups, just ahead of each
        # group's jb range, so the first exp fires ~20us earlier and the
        # ramp overlaps the x DMA stream.
        for tch in range(2):
            emit_kt(0, tch)
        emit_q_ob(0, 0)
        unit0_filler = [
            [(emit_kt, (0, tch)), (emit_kt, (0, tch + 1))]
            for tch in range(2, c.NTCH, 2)
        ]
        deferred = [(emit_kt, (ob, tch))
                    for ob in range(1, c.OB) for tch in range(c.NTCH)]
        deferred += [(emit_v, (tch, tbl))
                     for tch in range(c.NTCH) for tbl in range(c.TCH // P)]
        # all of V must be EMITTED before the first AV fires (unit AVD):
        # Tile cannot make an earlier-emitted read wait on a later write.
        n_drain_units = AVD
        drain_per_unit = -(-len(deferred) // n_drain_units)

        units = [(ic, hp) for ic in range(c.NIC) for hp in range(c.OB)]
        pend_av = []
        fin_queue = []

        def pop_av_gen():
            icav, hpav, e_pair = pend_av.pop(0)
            if hpav == c.OB - 1:
                fin_queue.extend([(icav, 0), (icav, 1)])
            return make_av(icav, hpav, e_pair)

        DONE = object()
        for u, (ic, hp) in enumerate(units):
            e_pair, sgen = make_scores(ic, hp)
            pend_av.append((ic, hp, e_pair))
            avgen = pop_av_gen() if u >= AVD else None
            # zip: 1 scores step (2 long-stream MMs) per 2 AV steps (4
            # LDW-bound MMs) keeps the PE weight path saturated under the
            # stream path.  In unit 0, the filler slots instead emit the
            # remaining kT[ob0] pieces at each group boundary, just ahead
            # of the jb range the next group contracts over.
            steps = 0
            while True:
                s = next(sgen, DONE)
                if s is not DONE:
                    steps += 1
                    if u == 0 and steps % 4 == 0 and unit0_filler:
                        for fn, args in unit0_filler.pop(0):
                            fn(*args)
                a = DONE
                if avgen is not None:
                    a = next(avgen, DONE)
                    if a is not DONE:
                        a = next(avgen, DONE)
                if s is DONE and (avgen is None or a is DONE):
                    break
            if u == 0:
                for ob in range(1, c.OB):
                    emit_q_ob(0, ob)
            if fin_queue:
                emit_finalize(*fin_queue.pop(0))
            # deferred kT/V pieces drain AFTER the unit's scores so they
            # never delay the exp stream (they are only needed by LATER
            # units: kT[ob h] by unit h, V by the first AV at unit AVD)
            for _ in range(drain_per_unit):
                if deferred:
                    fn, args = deferred.pop(0)
                    fn(*args)
            # spread the next chunk's qT projection one o-block per unit,
            # so it never opens a bubble in ScalarE's exp stream
            if ic + 1 < c.NIC:
                emit_q_ob(ic + 1, hp)
        while pend_av:
            for _ in pop_av_gen():
                pass
            if fin_queue:
                emit_finalize(*fin_queue.pop(0))
        for f in fin_queue:
            emit_finalize(*f)


def build_nc(cfg: Cfg = Cfg(), reps: int = 1):
    nc = bacc.Bacc()
    xT = nc.declare_dram_parameter("xT", [cfg.DIM, cfg.T], BF16, isOutput=False)
    wq = nc.declare_dram_parameter("wq", [cfg.DIM, cfg.OD], BF16, isOutput=False)
    wk = nc.declare_dram_parameter("wk", [cfg.DIM, cfg.OD], BF16, isOutput=False)
    wv = nc.declare_dram_parameter("wv", [cfg.DIM, cfg.OD], BF16, isOutput=False)
    woT = nc.declare_dram_parameter("woT", [cfg.OD, cfg.DIM], BF16, isOutput=False)
    out = nc.declare_dram_parameter("out", [cfg.T, cfg.DIM], F32, isOutput=True)
    with tile.TileContext(nc) as tc:
        for _ in range(reps):
            _emit_kernel(tc, cfg, xT[:], wq[:], wk[:], wv[:], woT[:], out[:])
    nc.finalize()
    return nc


def prepare_core_inputs(x, w_qkv, w_out, b, g, cfg: Cfg, n_groups: int):
    """Host-side shard prep for core (batch b, head-group g)."""
    H = cfg.NH * n_groups
    d = np.arange(cfg.DH)
    heads = np.arange(cfg.NH * g, cfg.NH * (g + 1))
    # w_qkv row for (k, head h, dim d) is d*(3*H) + k*H + h
    def gather(k_idx):
        rows = (d[None, :] * (3 * H) + k_idx * H + heads[:, None]).reshape(-1)
        return np.ascontiguousarray(w_qkv[rows, :].T, dtype=np.float32)

    return {
        "xT": np.ascontiguousarray(x[b].T).astype(NP_BF16),
        "wq": gather(0).astype(NP_BF16),
        "wk": gather(1).astype(NP_BF16),
        "wv": gather(2).astype(NP_BF16),
        "woT": np.ascontiguousarray(
            w_out[:, cfg.OD * g : cfg.OD * (g + 1)].T, dtype=np.float32
        ).astype(NP_BF16),
    }


_NC_CACHE = {}


def _get_nc(cfg: Cfg):
    if cfg not in _NC_CACHE:
        _NC_CACHE[cfg] = build_nc(cfg)
    return _NC_CACHE[cfg]


def run(x, w_qkv, w_out, b_out, trace=False):
    """Shard, execute on 8 cores, gather. Returns (out, BassKernelResults)."""
    cfg = Cfg()
    B, T, DIM = x.shape
    assert (T, DIM) == (cfg.T, cfg.DIM), (x.shape, cfg)
    n_groups = 2
    nc = _get_nc(cfg)
    in_maps = [
        prepare_core_inputs(x, w_qkv, w_out, b, g, cfg, n_groups)
        for b in range(B)
        for g in range(n_groups)
    ]
    res = run_bass_kernel_spmd(
        nc, in_maps, core_ids=list(range(len(in_maps))), trace=trace
    )
    out = np.empty((B, T, DIM), dtype=np.float32)
    for b in range(B):
        out[b] = res.results[2 * b]["out"] + res.results[2 * b + 1]["out"]
    out += b_out.astype(np.float32)
    return out, res


def _make_pjrt_fn(nc, in_maps):
    """Build a non-donating jitted 8-core runner for a prebuilt nc."""
    import jax
    import numpy as np_
    from jax.sharding import Mesh, PartitionSpec
    from jax.experimental.shard_map import shard_map

    from concourse import bass2jax

    bass2jax.install_neuronx_cc_hook()
    n_cores = len(in_maps)
    partition_name = nc.partition_id_tensor.name if nc.partition_id_tensor else None
    in_names, out_names, out_avals, zero_outs = [], [], [], []
    for alloc in nc.m.functions[0].allocations:
        if not isinstance(alloc, mybir.MemoryLocationSet):
            continue
        name = alloc.memorylocations[0].name
        if alloc.kind == "ExternalInput":
            if name != partition_name:
                in_names.append(name)
        elif alloc.kind == "ExternalOutput":
            shape = tuple(alloc.tensor_shape)
            dtype = mybir.dt.np(alloc.dtype)
            out_names.append(name)
            out_avals.append(jax.core.ShapedArray(shape, dtype))
            zero_outs.append(np_.zeros(shape, dtype))
    n_params = len(in_names)
    all_in_names = in_names + out_names
    if partition_name is not None:
        all_in_names = all_in_names + [partition_name]

    def _body(*args):
        operands = list(args)
        if partition_name is not None:
            operands.append(bass2jax.partition_id_tensor())
        return tuple(
            bass2jax._bass_exec_p.bind(
                *operands,
                out_avals=tuple(out_avals),
                in_names=tuple(all_in_names),
                out_names=tuple(out_names),
                lowering_input_output_aliases=(),
                sim_require_finite=True,
                sim_require_nnan=True,
                nc=nc,
            )
        )

    devices = jax.devices()[:n_cores]
    mesh = Mesh(np_.asarray(devices), ("core",))
    nin = n_params + len(out_names)
    f = jax.jit(
        shard_map(
            _body,
            mesh=mesh,
            in_specs=(PartitionSpec("core"),) * nin,
            out_specs=(PartitionSpec("core"),) * len(out_names),
            check_rep=False,
        ),
        keep_unused=True,
    )
    concat_in = [
        np_.concatenate([np_.asarray(in_maps[c][n]) for c in range(n_cores)], axis=0)
        for n in in_names
    ] + [np_.zeros((n_cores * z.shape[0], *z.shape[1:]), z.dtype) for z in zero_outs]
    dev_in = jax.device_put(concat_in)
    return f, dev_in


def _time_fn(f, dev_in, calls=8, rounds=8):
    import time

    import jax

    r = f(*dev_in)
    jax.block_until_ready(r)
    best = float("inf")
    for _ in range(rounds):
        t0 = time.perf_counter()
        rs = [f(*dev_in) for _ in range(calls)]
        jax.block_until_ready(rs)
        best = min(best, (time.perf_counter() - t0) / calls)
    return best


def time_hw(x, w_qkv, w_out, b_out, reps=(4, 100), pairs=2):
    """Marginal-cost HW timing: per-call time of an R2-repeat NEFF minus an
    R1-repeat NEFF, over (R2-R1), cancels the axon dispatch overhead.
    Measures the A/B pair `pairs` times; returns (tA, [estimates...])."""
    cfg = Cfg()
    B = x.shape[0]
    in_maps = [
        prepare_core_inputs(x, w_qkv, w_out, b, g, cfg, 2)
        for b in range(B)
        for g in range(2)
    ]
    r1, r2 = reps
    ncA = build_nc(cfg, reps=r1)
    fA, devA = _make_pjrt_fn(ncA, in_maps)
    ncB = build_nc(cfg, reps=r2)
    fB, devB = _make_pjrt_fn(ncB, in_maps)
    estimates = []
    tA = None
    for _ in range(pairs):
        tA = _time_fn(fA, devA)
        tB = _time_fn(fB, devB)
        estimates.append((tB - tA) / (r2 - r1))
    return tA, estimates


def kernel(x, w_qkv, w_out, b_out):
    x = np.asarray(x, dtype=np.float32)
    w_qkv = np.asarray(w_qkv, dtype=np.float32)
    w_out = np.asarray(w_out, dtype=np.float32)
    b_out = np.asarray(b_out, dtype=np.float32)
    try:
        out, _ = run(x, w_qkv, w_out, b_out, trace=False)
    except Exception:
        # one retry for transient device errors
        out, _ = run(x, w_qkv, w_out, b_out, trace=False)
    return out

